# revision 12
# baseline (speedup 1.0000x reference)
"""Trainium2 Bass kernel for nn_MultiHeadAttention_82454782148911.

Fused LN + QKV projection + causal/padded attention + output projection,
sharded over 8 NeuronCores: data-parallel over batch (B=2) x tensor-parallel
over heads (16 heads -> 4 per core).

Self-contained: hardcodes all shapes; builds one SPMD NEFF and runs it via
run_bass_kernel_spmd on cores 0-7.
"""
import math

import numpy as np

import concourse.bacc as bacc
import concourse.bass as bass
import concourse.mybir as mybir
import concourse.tile as tile
import concourse.bass_utils as bass_utils
from concourse.masks import make_upper_triangular

# problem shapes
B, S, D, H, DQ, DV = 2, 2048, 1024, 16, 64, 64
LN_EPS = 1e-5
N_CORES = 8
HPC = H // 4          # 4 heads per core
HD = HPC * DQ         # 256 head-dims per core
NEG = -1e30

F32 = mybir.dt.float32
F32R = mybir.dt.float32r
BF16 = mybir.dt.bfloat16
NP_BF16 = mybir.dt.np(BF16)

KT_TILES = S // 128   # 16
DC = D // 128         # 8 contraction chunks


def ceil_div(a, b):
    return (a + b - 1) // b


def build_nc():
    """Build the SPMD Bass module (identical program on all 8 cores)."""
    nc = bacc.Bacc("TRN2", target_bir_lowering=False, debug=False,
                   num_devices=N_CORES)

    # ---- dram io ----
    xs = {n: nc.dram_tensor(n, [S, D], F32, kind="ExternalInput")
          for n in ("xq", "xk", "xv")}
    ws = {n: nc.dram_tensor(n, [D, HD], BF16, kind="ExternalInput")
          for n in ("wq", "wk", "wv")}
    bws = {n: nc.dram_tensor(n, [1, HD], BF16, kind="ExternalInput")
           for n in ("bwq", "bwk", "bwv")}
    wo_h = nc.dram_tensor("wo", [HD, D], BF16, kind="ExternalInput")
    padrow_h = nc.dram_tensor("padrow", [1, S], BF16, kind="ExternalInput")
    attn_h = nc.dram_tensor("attn", [HPC, S, S], F32, kind="ExternalOutput")
    outp_h = nc.dram_tensor("outp", [S, D], F32, kind="ExternalOutput")

    with tile.TileContext(nc, num_cores=N_CORES) as tc:
        build_body(tc, xs, ws, bws, wo_h, padrow_h, attn_h, outp_h)
    nc.compile()
    return nc


def build_body(tc, xs, ws, bws, wo_h, padrow_h, attn_h, outp_h):
    nc = tc.nc
    MM = nc.tensor.matmul

    with tc.tile_pool(name="consts", bufs=1) as consts:
        ones_row = consts.tile([1, S], BF16)
        nc.vector.memset(ones_row, 1.0)
        # side A additive mask on PSUM scores: -1e30 strictly above diagonal
        triA = consts.tile([128, 128], F32)
        make_upper_triangular(nc, triA, val=NEG, diag=False)
        # side B multiplicative mask on exp(S.T): 1 at k<=q (upper incl diag)
        triB = consts.tile([128, 128], BF16)
        make_upper_triangular(nc, triB, val=1.0, diag=True)
        eps_t = consts.tile([128, 1], F32)
        nc.vector.memset(eps_t, LN_EPS)
        wo_sb = consts.tile([128, 2, D], BF16)
        nc.sync.dma_start(out=wo_sb, in_=wo_h.ap().rearrange("(a p) d -> p a d", p=128))
        bw_sb = {}
        for n in ("bwq", "bwk", "bwv"):
            bw_sb[n] = consts.tile([1, HD], BF16, tag=n, name=n)
            nc.sync.dma_start(out=bw_sb[n], in_=bws[n].ap())

        with tc.tile_pool(name="qkv", bufs=1) as qkvp:
            QT = [qkvp.tile([65, S], BF16, tag=f"qt{h}", name=f"qt{h}") for h in range(HPC)]
            KT = [qkvp.tile([65, S], BF16, tag=f"kt{h}", name=f"kt{h}") for h in range(HPC)]
            vaug = qkvp.tile([128, KT_TILES, HPC * 65], BF16, tag="vaug")
            ctxT = qkvp.tile([128, 2, S], BF16, tag="ctxt")
            for h in range(HPC):
                nc.vector.memset(QT[h][64:65, :], 1.0)
                nc.sync.dma_start(out=KT[h][64:65, :], in_=padrow_h.ap())
                nc.vector.memset(vaug[:, :, 65 * h + 64:65 * h + 65], 1.0)

            # ============ phase 1: LN + transpose + projections ============
            def ln_and_transpose(ph1, xpool, name, xlnT):
                """x (dram f32) -> LN -> bf16 -> transposed into xlnT."""
                x_r = xs[name].ap().rearrange("(t p) d -> p t d", p=128)
                st = ph1.tile([128, KT_TILES, 2, 6], F32, tag="st")
                mv = ph1.tile([128, KT_TILES, 2], F32, tag="mv")
                negmu = ph1.tile([128, KT_TILES], F32, tag="nm")
                rstd = ph1.tile([128, KT_TILES], F32, tag="rs")
                xbig = xpool.tile([128, KT_TILES, D], BF16, tag="xbig")
                for tt in range(KT_TILES):
                    nc.gpsimd.dma_start(out=xbig[:, tt, :], in_=x_r[:, tt, :])  # cast f32->bf16
                    nc.vector.bn_stats(out=st[:, tt, 0, :], in_=xbig[:, tt, 0:512])
                    nc.vector.bn_stats(out=st[:, tt, 1, :], in_=xbig[:, tt, 512:1024])
                    nc.vector.bn_aggr(out=mv[:, tt, :], in_=st[:, tt, :, :])
                # batched stats post-processing
                nc.vector.tensor_scalar_mul(negmu, mv[:, :, 0], -1.0)
                nc.scalar.activation(out=rstd, in_=mv[:, :, 1],
                                     func=mybir.ActivationFunctionType.Sqrt,
                                     bias=eps_t, scale=1.0)
                nc.vector.reciprocal(rstd, rstd)
                for tt in range(KT_TILES):
                    xln = ph1.tile([128, D], BF16, tag="xln")
                    nc.vector.tensor_scalar(
                        out=xln, in0=xbig[:, tt, :],
                        scalar1=negmu[:, tt:tt + 1], scalar2=rstd[:, tt:tt + 1],
                        op0=mybir.AluOpType.add, op1=mybir.AluOpType.mult)
                    nc.sync.dma_start_transpose(xlnT[:, tt], xln)

            with tc.tile_pool(name="ph1", bufs=3) as ph1:
                # --- V first (so xlnT_v's pool can be released early) ---
                wv_sb = ph1.tile([128, DC, HD], BF16, tag="wv")
                nc.sync.dma_start(out=wv_sb, in_=ws["wv"].ap().rearrange("(c p) m -> p c m", p=128))
                with tc.tile_pool(name="ph1v", bufs=1) as ph1v, \
                     tc.tile_pool(name="psv", bufs=2, space="PSUM") as psv_pool:
                    xlnT_v = ph1v.tile([128, KT_TILES, DC, 128], BF16, tag="xlnT_v")
                    with tc.tile_pool(name="xb_v", bufs=1) as xpool:
                        ln_and_transpose(ph1, xpool, "xv", xlnT_v)
                    for tt in range(KT_TILES):
                        psv = psv_pool.tile([128, HD], F32, tag="pv")
                        for c in range(DC):
                            MM(psv, lhsT=xlnT_v[:, tt, c, :], rhs=wv_sb[:, c, :],
                               start=(c == 0), stop=False)
                        MM(psv, lhsT=ones_row[0:1, 0:128], rhs=bw_sb["bwv"],
                           start=False, stop=True)
                        for h in range(HPC):
                            nc.vector.tensor_copy(
                                out=vaug[:, tt, 65 * h:65 * h + 64],
                                in_=psv[:, 64 * h:64 * h + 64])

                # --- Q.T / K.T ---
                with tc.tile_pool(name="ph1qk", bufs=1) as ph1qk, \
                     tc.tile_pool(name="psj", bufs=3, space="PSUM") as psj_pool:
                    for name, wname in (("xq", "wq"), ("xk", "wk")):
                        w_sb = ph1.tile([128, DC, HD], BF16, tag=f"w_{wname}")
                        nc.sync.dma_start(out=w_sb, in_=ws[wname].ap().rearrange("(c p) m -> p c m", p=128))
                        xlnT = ph1qk.tile([128, KT_TILES, DC, 128], BF16, tag=f"xlnT_{name}")
                        with tc.tile_pool(name=f"xb_{name}", bufs=1) as xpool:
                            ln_and_transpose(ph1, xpool, name, xlnT)
                        dst = QT if name == "xq" else KT
                        bwn = "bwq" if name == "xq" else "bwk"
                        for m in range(2):
                            for qc in range(4):
                                ps = psj_pool.tile([128, 512], F32, tag="pj")
                                for c in range(DC):
                                    MM(ps, lhsT=w_sb[:, c, 128 * m:128 * m + 128],
                                       rhs=xlnT[:, 4 * qc:4 * qc + 4, c, :],
                                       start=(c == 0), stop=False)
                                MM(ps, lhsT=bw_sb[bwn][0:1, 128 * m:128 * m + 128],
                                   rhs=ones_row[0:1, 512 * qc:512 * qc + 512],
                                   start=False, stop=True)
                                for i in range(2):
                                    h = 2 * m + i
                                    nc.vector.tensor_copy(
                                        out=dst[h][0:64, 512 * qc:512 * qc + 512],
                                        in_=ps[64 * i:64 * i + 64, :])

            # ============ phase 2: attention ============
            with tc.tile_pool(name="ph2", bufs=2) as ph2, \
                 tc.tile_pool(name="php", bufs=3) as php, \
                 tc.tile_pool(name="psS", bufs=2, space="PSUM") as psS, \
                 tc.tile_pool(name="psC", bufs=2, space="PSUM") as psC:
                for h in range(HPC):
                    # ---- side B: S.T tiles -> exp -> masked (bf16) ----
                    est = []
                    for kt in range(KT_TILES):
                        w = S - 128 * kt
                        e = ph2.tile([128, w], BF16, tag=f"est{kt}", name=f"est{kt}")
                        for ci in range(ceil_div(w, 1024)):
                            cw = min(1024, w - 1024 * ci)
                            pst = psS.tile([128, 1024], F32, tag="st")
                            for i in range(ceil_div(cw, 512)):
                                n = min(512, cw - 512 * i)
                                q0 = 128 * kt + 1024 * ci + 512 * i
                                MM(pst[:, 512 * i:512 * i + n],
                                   lhsT=KT[h][:, 128 * kt:128 * kt + 128],
                                   rhs=QT[h][:, q0:q0 + n],
                                   start=True, stop=True)
                            nc.scalar.activation(
                                out=e[:, 1024 * ci:1024 * ci + cw],
                                in_=pst[:, 0:cw],
                                func=mybir.ActivationFunctionType.Exp,
                                scale=1.0 / math.sqrt(DQ))
                        # causal mask on the diagonal 128x128 block (cols q in
                        # [128kt,128kt+128) are the first 128 cols of e)
                        nc.vector.tensor_mul(e[:, 0:128], e[:, 0:128], triB)
                        est.append(e)

                    # ---- phase C: ctx.T (+softmax sum) and normalize ----
                    for qc in range(4):
                        pc = psC.tile([65, 512], F32, tag="ctx")
                        last = 4 * qc + 3
                        for kt in range(last + 1):
                            off = 512 * qc - 128 * kt
                            if off >= 0:
                                # est tile covers the whole q-chunk
                                MM(pc, lhsT=vaug[:, kt, 65 * h:65 * h + 65],
                                   rhs=est[kt][:, off:off + 512],
                                   start=(kt == 0), stop=(kt == last))
                            else:
                                # tile starts inside the chunk (diag boundary):
                                # columns q < 128*kt of this chunk get no
                                # contribution from this k-tile (causal zeros)
                                MM(pc[:, -off:512],
                                   lhsT=vaug[:, kt, 65 * h:65 * h + 65],
                                   rhs=est[kt][:, 0:512 + off],
                                   start=False, stop=(kt == last))
                        rl = ph2.tile([1, 512], F32, tag="rl")
                        nc.vector.reciprocal(rl, pc[64:65, :])
                        rlb_sb = ph2.tile([64, 512], F32, tag="rlb_sb")
                        nc.gpsimd.partition_broadcast(rlb_sb, rl)
                        nc.vector.tensor_mul(
                            ctxT[64 * (h % 2):64 * (h % 2) + 64, h // 2,
                                 512 * qc:512 * qc + 512],
                            pc[0:64, :], rlb_sb)

                    # ---- side A: normalized P rows -> HBM ----
                    for t in range(KT_TILES):
                        wa = 128 * (t + 1)
                        nch = ceil_div(wa, 1024)
                        Pt = php.tile([128, S], BF16, tag="P")
                        lp = php.tile([128, 2], F32, tag="lp")
                        for ci in range(nch):
                            cw = min(1024, wa - 1024 * ci)
                            ps = psS.tile([128, 1024], F32, tag="st")
                            for i in range(ceil_div(cw, 512)):
                                n = min(512, cw - 512 * i)
                                k0 = 1024 * ci + 512 * i
                                MM(ps[:, 512 * i:512 * i + n],
                                   lhsT=QT[h][:, 128 * t:128 * t + 128],
                                   rhs=KT[h][:, k0:k0 + n],
                                   start=True, stop=True)
                            if ci == nch - 1:
                                # additive causal mask on diagonal block
                                nc.vector.tensor_add(ps[:, cw - 128:cw],
                                                     ps[:, cw - 128:cw], triA)
                            nc.scalar.activation(
                                out=Pt[:, 1024 * ci:1024 * ci + cw],
                                in_=ps[:, 0:cw],
                                func=mybir.ActivationFunctionType.Exp,
                                scale=1.0 / math.sqrt(DQ),
                                accum_out=lp[:, ci:ci + 1])
                        lsum = php.tile([128, 1], F32, tag="ls")
                        if nch == 2:
                            nc.vector.tensor_add(lsum, lp[:, 0:1], lp[:, 1:2])
                        else:
                            nc.vector.tensor_copy(lsum, lp[:, 0:1])
                        rlp = php.tile([128, 1], F32, tag="rlp")
                        nc.vector.reciprocal(rlp, lsum)
                        nc.vector.tensor_scalar_mul(Pt[:, 0:wa], Pt[:, 0:wa], rlp)
                        nc.gpsimd.dma_start(
                            out=attn_h.ap()[h, 128 * t:128 * t + 128, 0:wa],
                            in_=Pt[:, 0:wa])  # cast bf16->f32

            # ============ phase 3: output projection ============
            with tc.tile_pool(name="ph3", bufs=3) as ph3, \
                 tc.tile_pool(name="psW", bufs=2, space="PSUM") as psW:
                for t in range(KT_TILES):
                    ot = ph3.tile([128, D], F32, tag="ot")
                    for nn in range(2):
                        pw = psW.tile([128, 512], F32, tag="w")
                        for a in range(2):
                            MM(pw, lhsT=ctxT[:, a, 128 * t:128 * t + 128],
                               rhs=wo_sb[:, a, 512 * nn:512 * nn + 512],
                               start=(a == 0), stop=(a == 1))
                        nc.any.tensor_copy(out=ot[:, 512 * nn:512 * nn + 512], in_=pw)
                    nc.sync.dma_start(out=outp_h.ap()[128 * t:128 * t + 128, :], in_=ot)



_NC_CACHE = None


def get_nc():
    global _NC_CACHE
    if _NC_CACHE is None:
        _NC_CACHE = build_nc()
    return _NC_CACHE


def make_in_maps(query_sequences, key_sequences, value_sequences,
                 key_value_sequence_lengths, ln_gamma, ln_beta,
                 Wq, bq, Wk, bk, Wv, bv, Wo, bo):
    g = np.asarray(ln_gamma, np.float32)
    be = np.asarray(ln_beta, np.float32)
    in_maps = []
    for c in range(N_CORES):
        b = c // 4
        hg = c % 4
        cs = slice(HD * hg, HD * hg + HD)   # head-dim columns of this core
        length = int(np.asarray(key_value_sequence_lengths)[b])
        padrow = np.zeros((1, S), np.float32)
        padrow[0, length:] = NEG
        m = {
            "xq": np.ascontiguousarray(np.asarray(query_sequences, np.float32)[b]),
            "xk": np.ascontiguousarray(np.asarray(key_sequences, np.float32)[b]),
            "xv": np.ascontiguousarray(np.asarray(value_sequences, np.float32)[b]),
            "wq": (g[:, None] * np.asarray(Wq, np.float32)[:, cs]).astype(NP_BF16),
            "wk": (g[:, None] * np.asarray(Wk, np.float32)[:, cs]).astype(NP_BF16),
            "wv": (g[:, None] * np.asarray(Wv, np.float32)[:, cs]).astype(NP_BF16),
            "bwq": (be @ np.asarray(Wq, np.float32)[:, cs] + np.asarray(bq, np.float32)[cs]).reshape(1, HD).astype(NP_BF16),
            "bwk": (be @ np.asarray(Wk, np.float32)[:, cs] + np.asarray(bk, np.float32)[cs]).reshape(1, HD).astype(NP_BF16),
            "bwv": (be @ np.asarray(Wv, np.float32)[:, cs] + np.asarray(bv, np.float32)[cs]).reshape(1, HD).astype(NP_BF16),
            "wo": np.ascontiguousarray(np.asarray(Wo, np.float32)[cs, :]).astype(NP_BF16),
            "padrow": padrow.astype(NP_BF16),
        }
        in_maps.append(m)
    return in_maps


def kernel(**inputs):
    nc = get_nc()
    in_maps = make_in_maps(**inputs)
    res = bass_utils.run_bass_kernel_spmd(nc, in_maps, core_ids=list(range(N_CORES)))
    out = np.zeros((B, S, D), np.float32)
    attn = np.zeros((B * H, S, S), np.float32)
    bo = np.asarray(inputs["bo"], np.float32)
    for c in range(N_CORES):
        b = c // 4
        hg = c % 4
        out[b] += res.results[c]["outp"]
        attn[16 * b + 4 * hg:16 * b + 4 * hg + HPC] = res.results[c]["attn"]
    out += bo[None, None, :]
    return out, attn


# revision 16
# speedup vs baseline: 1.1253x; 1.1253x over previous
"""Trainium2 Bass kernel for nn_MultiHeadAttention_82454782148911.

Fused LN + QKV projection + causal/padded attention + output projection,
sharded over 8 NeuronCores: data-parallel over batch (B=2) x tensor-parallel
over heads (16 heads -> 4 per core).

Self-contained: hardcodes all shapes; builds one SPMD NEFF and runs it via
run_bass_kernel_spmd on cores 0-7.
"""
import math

import numpy as np

import concourse.bacc as bacc
import concourse.bass as bass
import concourse.mybir as mybir
import concourse.tile as tile
import concourse.bass_utils as bass_utils
from concourse.masks import make_upper_triangular

# problem shapes
B, S, D, H, DQ, DV = 2, 2048, 1024, 16, 64, 64
LN_EPS = 1e-5
N_CORES = 8
HPC = H // 4          # 4 heads per core
HD = HPC * DQ         # 256 head-dims per core
NEG = -1e30

F32 = mybir.dt.float32
BF16 = mybir.dt.bfloat16
NP_BF16 = mybir.dt.np(BF16)

KT_TILES = S // 128   # 16
DC = D // 128         # 8 contraction chunks

# which phases to build (for ablation timing): subset of {"B","C","A","W"}
PHASES = {"B", "C", "A", "W"}


def ceil_div(a, b):
    return (a + b - 1) // b


def build_nc():
    """Build the SPMD Bass module (identical program on all 8 cores)."""
    nc = bacc.Bacc("TRN2", target_bir_lowering=False, debug=False,
                   num_devices=N_CORES)

    # ---- dram io ----
    xs = {n: nc.dram_tensor(n, [S, D], BF16, kind="ExternalInput")
          for n in ("xq", "xk", "xv")}
    ws = {n: nc.dram_tensor(n, [D, HD], BF16, kind="ExternalInput")
          for n in ("wq", "wk", "wv")}
    bws = {n: nc.dram_tensor(n, [1, HD], BF16, kind="ExternalInput")
           for n in ("bwq", "bwk", "bwv")}
    wo_h = nc.dram_tensor("wo", [HD, D], BF16, kind="ExternalInput")
    padrow_h = nc.dram_tensor("padrow", [1, S], BF16, kind="ExternalInput")
    attn_h = nc.dram_tensor("attn", [HPC, S, S], F32, kind="ExternalOutput")
    outp_h = nc.dram_tensor("outp", [S, D], F32, kind="ExternalOutput")

    with tile.TileContext(nc, num_cores=N_CORES) as tc:
        build_body(tc, xs, ws, bws, wo_h, padrow_h, attn_h, outp_h)
    nc.compile()
    return nc


def build_body(tc, xs, ws, bws, wo_h, padrow_h, attn_h, outp_h):
    nc = tc.nc
    MM = nc.tensor.matmul
    AF = mybir.ActivationFunctionType

    with tc.tile_pool(name="consts", bufs=1) as consts:
        ones_row = consts.tile([1, S], BF16)
        nc.vector.memset(ones_row, 1.0)
        # side A additive mask on PSUM scores: -1e30 strictly above diagonal
        triA = consts.tile([128, 128], F32)
        make_upper_triangular(nc, triA, val=NEG, diag=False)
        # side B multiplicative mask on exp(S.T): 1 at k<=q (upper incl diag)
        triB = consts.tile([128, 128], BF16)
        make_upper_triangular(nc, triB, val=1.0, diag=True)
        eps_t = consts.tile([128, 1], F32)
        nc.vector.memset(eps_t, LN_EPS)
        wo_sb = consts.tile([128, 2, D], BF16)
        nc.sync.dma_start(out=wo_sb, in_=wo_h.ap().rearrange("(a p) d -> p a d", p=128))
        bw_sb = {}
        for n in ("bwq", "bwk", "bwv"):
            bw_sb[n] = consts.tile([1, HD], BF16, tag=n, name=n)
            nc.sync.dma_start(out=bw_sb[n], in_=bws[n].ap())
        w_sb = {}
        for n in ("wq", "wk", "wv"):
            w_sb[n] = consts.tile([128, DC, HD], BF16, tag=f"w{n}", name=f"w{n}")
            nc.sync.dma_start(out=w_sb[n], in_=ws[n].ap().rearrange("(c p) m -> p c m", p=128))

        with tc.tile_pool(name="qkv", bufs=1) as qkvp:
            QT = [qkvp.tile([65, S], BF16, tag=f"qt{h}", name=f"qt{h}") for h in range(HPC)]
            KT = [qkvp.tile([65, S], BF16, tag=f"kt{h}", name=f"kt{h}") for h in range(HPC)]
            vaug = qkvp.tile([128, KT_TILES, HPC * 65], BF16, tag="vaug")
            ctxT = qkvp.tile([128, 2, S], BF16, tag="ctxt")
            for h in range(HPC):
                nc.vector.memset(QT[h][64:65, :], 1.0)
                nc.sync.dma_start(out=KT[h][64:65, :], in_=padrow_h.ap())
                nc.vector.memset(vaug[:, :, 65 * h + 64:65 * h + 65], 1.0)

            # ========= phase 1: LN + transpose + projections, by quartet =========
            with tc.tile_pool(name="px", bufs=2) as px, \
                 tc.tile_pool(name="pxt", bufs=2) as pxt, \
                 tc.tile_pool(name="pstats", bufs=3) as pstats, \
                 tc.tile_pool(name="psv", bufs=2, space="PSUM") as psv_pool, \
                 tc.tile_pool(name="psj", bufs=3, space="PSUM") as psj_pool:

                def quartet(name, g):
                    """Load+LN 4 token-tiles of tensor `name`; returns the
                    transposed group tile [128, 4, DC, 128]."""
                    x_r = xs[name].ap().rearrange("(t p) d -> p t d", p=128)
                    x4 = px.tile([128, 4, D], BF16, tag=f"x4_{name}", name=f"x4{name}{g}")
                    nc.sync.dma_start(out=x4, in_=x_r[:, 4 * g:4 * g + 4, :])
                    st = pstats.tile([128, 4, 2, 6], F32, tag=f"st_{name}", name=f"st{name}{g}")
                    mv = pstats.tile([128, 4, 2], F32, tag=f"mv_{name}", name=f"mv{name}{g}")
                    negmu = pstats.tile([128, 4], F32, tag=f"nm_{name}", name=f"nm{name}{g}")
                    rstd = pstats.tile([128, 4], F32, tag=f"rs_{name}", name=f"rs{name}{g}")
                    for i in range(4):
                        nc.vector.bn_stats(out=st[:, i, 0, :], in_=x4[:, i, 0:512])
                        nc.vector.bn_stats(out=st[:, i, 1, :], in_=x4[:, i, 512:1024])
                        nc.vector.bn_aggr(out=mv[:, i, :], in_=st[:, i, :, :])
                    nc.vector.tensor_scalar_mul(negmu, mv[:, :, 0], -1.0)
                    nc.scalar.activation(out=rstd, in_=mv[:, :, 1], func=AF.Sqrt,
                                         bias=eps_t, scale=1.0)
                    nc.vector.reciprocal(rstd, rstd)
                    for i in range(4):
                        nc.vector.tensor_scalar(
                            out=x4[:, i, :], in0=x4[:, i, :],
                            scalar1=negmu[:, i:i + 1], scalar2=rstd[:, i:i + 1],
                            op0=mybir.AluOpType.add, op1=mybir.AluOpType.mult)
                    xt = pxt.tile([128, 4, DC, 128], BF16, tag=f"xt_{name}", name=f"xt{name}{g}")
                    nc.sync.dma_start_transpose(xt, x4.rearrange("p a d -> p (a d)"))
                    return xt

                for g in range(4):
                    xt_v = quartet("xv", g)
                    xt_q = quartet("xq", g)
                    xt_k = quartet("xk", g)
                    # V (natural layout) for this quartet's token tiles
                    for i in range(4):
                        tt = 4 * g + i
                        psv = psv_pool.tile([128, HD], F32, tag="pv")
                        for c in range(DC):
                            MM(psv, lhsT=xt_v[:, i, c, :], rhs=w_sb["wv"][:, c, :],
                               start=(c == 0), stop=False)
                        MM(psv, lhsT=ones_row[0:1, 0:128], rhs=bw_sb["bwv"],
                           start=False, stop=True)
                        for h in range(HPC):
                            nc.vector.tensor_copy(
                                out=vaug[:, tt, 65 * h:65 * h + 64],
                                in_=psv[:, 64 * h:64 * h + 64])
                    # Q.T / K.T for q-chunk g
                    for xt, dst, wname, bwn in ((xt_q, QT, "wq", "bwq"),
                                                (xt_k, KT, "wk", "bwk")):
                        for m in range(2):
                            ps = psj_pool.tile([128, 512], F32, tag="pj")
                            for c in range(DC):
                                MM(ps, lhsT=w_sb[wname][:, c, 128 * m:128 * m + 128],
                                   rhs=xt[:, :, c, :],
                                   start=(c == 0), stop=False)
                            MM(ps, lhsT=bw_sb[bwn][0:1, 128 * m:128 * m + 128],
                               rhs=ones_row[0:1, 512 * g:512 * g + 512],
                               start=False, stop=True)
                            for i in range(2):
                                h = 2 * m + i
                                nc.vector.tensor_copy(
                                    out=dst[h][0:64, 512 * g:512 * g + 512],
                                    in_=ps[64 * i:64 * i + 64, :])

            # ============ phase 2: attention ============
            with tc.tile_pool(name="ph2", bufs=2) as ph2, \
                 tc.tile_pool(name="php", bufs=3) as php, \
                 tc.tile_pool(name="psS", bufs=3, space="PSUM") as psS, \
                 tc.tile_pool(name="psC", bufs=2, space="PSUM") as psC:
                for h in range(HPC):
                    # ---- side B: S.T tiles -> exp -> masked (bf16) ----
                    if "B" not in PHASES:
                        break
                    est = []
                    for kt in range(KT_TILES):
                        w = S - 128 * kt
                        e = ph2.tile([128, w], BF16, tag=f"est{kt}", name=f"est{kt}")
                        for ci in range(ceil_div(w, 1024)):
                            cw = min(1024, w - 1024 * ci)
                            pstile = psS.tile([128, 1024], F32, tag="st")
                            for i in range(ceil_div(cw, 512)):
                                n = min(512, cw - 512 * i)
                                q0 = 128 * kt + 1024 * ci + 512 * i
                                MM(pstile[:, 512 * i:512 * i + n],
                                   lhsT=KT[h][:, 128 * kt:128 * kt + 128],
                                   rhs=QT[h][:, q0:q0 + n],
                                   start=True, stop=True)
                            nc.scalar.activation(
                                out=e[:, 1024 * ci:1024 * ci + cw],
                                in_=pstile[:, 0:cw],
                                func=AF.Exp, scale=1.0 / math.sqrt(DQ))
                        # causal mask on the diagonal 128x128 block (cols q in
                        # [128kt,128kt+128) are the first 128 cols of e)
                        nc.vector.tensor_mul(e[:, 0:128], e[:, 0:128], triB)
                        est.append(e)

                    # ---- phase C: ctx.T (+softmax sum) and normalize ----
                    rlT = ph2.tile([128, 16], F32, tag="rlT", name=f"rlT{h}")
                    lnlT = ph2.tile([128, 16], F32, tag="lnlT", name=f"lnlT{h}")
                    for qc in range(4 if "C" in PHASES else 0):
                        pc = psC.tile([65, 512], F32, tag="ctx")
                        last = 4 * qc + 3
                        for kt in range(last + 1):
                            off = 512 * qc - 128 * kt
                            if off >= 0:
                                MM(pc, lhsT=vaug[:, kt, 65 * h:65 * h + 65],
                                   rhs=est[kt][:, off:off + 512],
                                   start=(kt == 0), stop=(kt == last))
                            else:
                                # tile starts inside the chunk (causal boundary)
                                MM(pc[:, -off:512],
                                   lhsT=vaug[:, kt, 65 * h:65 * h + 65],
                                   rhs=est[kt][:, 0:512 + off],
                                   start=False, stop=(kt == last))
                        rl = ph2.tile([1, 512], F32, tag="rl")
                        nc.vector.reciprocal(rl, pc[64:65, :])
                        rlb_sb = ph2.tile([64, 512], F32, tag="rlb_sb")
                        nc.gpsimd.partition_broadcast(rlb_sb, rl)
                        # transpose 1/l into per-partition layout (tiny DMAs):
                        # rlT[p, 4qc+j] = rl[0, 128j + p]
                        for j in range(4):
                            nc.sync.dma_start(
                                out=rlT[:, 4 * qc + j:4 * qc + j + 1],
                                in_=rl[0:1, 128 * j:128 * j + 128])
                        nc.vector.tensor_mul(
                            ctxT[64 * (h % 2):64 * (h % 2) + 64, h // 2,
                                 512 * qc:512 * qc + 512],
                            pc[0:64, :], rlb_sb)
                    if "C" in PHASES:
                        # ln(1/l) = -ln(l): the softmax bias for side A
                        nc.scalar.activation(out=lnlT, in_=rlT, func=AF.Ln, scale=1.0)

                    # ---- side A: normalized P rows -> HBM (f32, HWDGE) ----
                    for t in range(KT_TILES if "A" in PHASES else 0):
                        wa = 128 * (t + 1)
                        nch = ceil_div(wa, 1024)
                        Pt = php.tile([128, S], F32, tag="P")
                        for ci in range(nch):
                            cw = min(1024, wa - 1024 * ci)
                            ps = psS.tile([128, 1024], F32, tag="st")
                            for i in range(ceil_div(cw, 512)):
                                n = min(512, cw - 512 * i)
                                k0 = 1024 * ci + 512 * i
                                MM(ps[:, 512 * i:512 * i + n],
                                   lhsT=QT[h][:, 128 * t:128 * t + 128],
                                   rhs=KT[h][:, k0:k0 + n],
                                   start=True, stop=True)
                            if ci == nch - 1:
                                # additive causal mask on diagonal block
                                nc.vector.tensor_add(ps[:, cw - 128:cw],
                                                     ps[:, cw - 128:cw], triA)
                            nc.scalar.activation(
                                out=Pt[:, 1024 * ci:1024 * ci + cw],
                                in_=ps[:, 0:cw],
                                func=AF.Exp, scale=1.0 / math.sqrt(DQ),
                                bias=lnlT[:, t:t + 1])
                        nc.sync.dma_start(
                            out=attn_h.ap()[h, 128 * t:128 * t + 128, 0:wa],
                            in_=Pt[:, 0:wa])

            # ============ phase 3: output projection ============
            with tc.tile_pool(name="ph3", bufs=3) as ph3, \
                 tc.tile_pool(name="psW", bufs=2, space="PSUM") as psW:
                for t in range(KT_TILES if "W" in PHASES else 0):
                    ot = ph3.tile([128, D], F32, tag="ot")
                    for nn in range(2):
                        pw = psW.tile([128, 512], F32, tag="w")
                        for a in range(2):
                            MM(pw, lhsT=ctxT[:, a, 128 * t:128 * t + 128],
                               rhs=wo_sb[:, a, 512 * nn:512 * nn + 512],
                               start=(a == 0), stop=(a == 1))
                        nc.any.tensor_copy(out=ot[:, 512 * nn:512 * nn + 512], in_=pw)
                    nc.sync.dma_start(out=outp_h.ap()[128 * t:128 * t + 128, :], in_=ot)


_NC_CACHE = None


def get_nc():
    global _NC_CACHE
    if _NC_CACHE is None:
        _NC_CACHE = build_nc()
    return _NC_CACHE


def make_in_maps(query_sequences, key_sequences, value_sequences,
                 key_value_sequence_lengths, ln_gamma, ln_beta,
                 Wq, bq, Wk, bk, Wv, bv, Wo, bo):
    g = np.asarray(ln_gamma, np.float32)
    be = np.asarray(ln_beta, np.float32)
    xq_b = np.asarray(query_sequences, np.float32).astype(NP_BF16)
    xk_b = np.asarray(key_sequences, np.float32).astype(NP_BF16)
    xv_b = np.asarray(value_sequences, np.float32).astype(NP_BF16)
    in_maps = []
    for c in range(N_CORES):
        b = c // 4
        hg = c % 4
        cs = slice(HD * hg, HD * hg + HD)   # head-dim columns of this core
        length = int(np.asarray(key_value_sequence_lengths)[b])
        padrow = np.zeros((1, S), np.float32)
        padrow[0, length:] = NEG
        m = {
            "xq": np.ascontiguousarray(xq_b[b]),
            "xk": np.ascontiguousarray(xk_b[b]),
            "xv": np.ascontiguousarray(xv_b[b]),
            "wq": (g[:, None] * np.asarray(Wq, np.float32)[:, cs]).astype(NP_BF16),
            "wk": (g[:, None] * np.asarray(Wk, np.float32)[:, cs]).astype(NP_BF16),
            "wv": (g[:, None] * np.asarray(Wv, np.float32)[:, cs]).astype(NP_BF16),
            "bwq": (be @ np.asarray(Wq, np.float32)[:, cs] + np.asarray(bq, np.float32)[cs]).reshape(1, HD).astype(NP_BF16),
            "bwk": (be @ np.asarray(Wk, np.float32)[:, cs] + np.asarray(bk, np.float32)[cs]).reshape(1, HD).astype(NP_BF16),
            "bwv": (be @ np.asarray(Wv, np.float32)[:, cs] + np.asarray(bv, np.float32)[cs]).reshape(1, HD).astype(NP_BF16),
            "wo": np.ascontiguousarray(np.asarray(Wo, np.float32)[cs, :]).astype(NP_BF16),
            "padrow": padrow.astype(NP_BF16),
        }
        in_maps.append(m)
    return in_maps


def kernel(**inputs):
    nc = get_nc()
    in_maps = make_in_maps(**inputs)
    res = bass_utils.run_bass_kernel_spmd(nc, in_maps, core_ids=list(range(N_CORES)))
    out = np.zeros((B, S, D), np.float32)
    attn = np.zeros((B * H, S, S), np.float32)
    bo = np.asarray(inputs["bo"], np.float32)
    for c in range(N_CORES):
        b = c // 4
        hg = c % 4
        out[b] += res.results[c]["outp"]
        attn[16 * b + 4 * hg:16 * b + 4 * hg + HPC] = res.results[c]["attn"]
    out += bo[None, None, :]
    return out, attn
